# revision 9
# baseline (speedup 1.0000x reference)
"""Trainium2 Bass kernel for GQA attention (B=2,S=2048,D=2048,H=16,KV=4,HD=128)
with RoPE + causal mask, sharded over 8 NeuronCores:
  2-way data parallel over batch x 4-way tensor parallel over KV groups.

Core c = (b, g): b = c // 4, g = c % 4.
Each core computes, for its batch b and KV group g (q heads 4g..4g+3):
  QT_h [HD,S], KT [HD,S] (RoPE'd), V [S,HD]    via matmul vs xT [D,S]
  scoresT [sk,sq] = KT_tile^T-free matmul, exp on ScalarE (scale folded),
  row-sums via ones-matmul, AV with V tiles stationary -> outT [HD,sq],
  per-head normalization via reciprocal + DMA partition-broadcast,
  partial y = attn_norm @ wo_rows[g]  -> host sums the 4 partials per batch.

Layout notes:
  matmul(out, lhsT, rhs) = lhsT.T @ rhs, contraction over the partition dim.
  All contractions are K=128.  Causality is exploited at block granularity:
  fully-masked (sk,sq) blocks are skipped; diagonal blocks add the true mask
  slice from the passed-in mask tensor.
"""

import os
import sys
from contextlib import ExitStack

import numpy as np

import concourse.bass as bass
import concourse.bacc as bacc
import concourse.mybir as mybir
import concourse.tile as tile
from concourse import bass_utils

# ---------------- problem constants (hardcoded per contract) ----------------
B, S, D = 2, 2048, 2048
H, KV, HD = 16, 4, 128
REP = H // KV            # 4 q heads per kv head
NG = KV                  # 4 tensor-parallel groups
NCORES = 8
THETA = 10000.0
SCALE = 1.0 / float(np.sqrt(HD))

P = 128                  # partition dim
SC = 512                 # moving free-dim chunk (fp32 max)
NDT = S // P             # 16 tiles of 128 along S or D
NCH = S // SC            # 4 chunks of 512 along S
NH = REP                 # 4 q-heads per core

FP32 = mybir.dt.float32
F32R = mybir.dt.float32r

# matmul dtype switch: "fp32" (exact, 4 cyc/row) or "f32r" (1 cyc/row @ N>=256)
MM_MODE = os.environ.get("KERNEL_MM_MODE", "fp32")

_CACHE = {}


def _mm_ap(ap):
    if MM_MODE == "f32r":
        return ap.bitcast(F32R)
    return ap


def _build_program():
    nc = bacc.Bacc("TRN2", target_bir_lowering=False, debug=False)

    xT_d = nc.dram_tensor("xT", [D, S], FP32, kind="ExternalInput").ap()
    wq_d = nc.dram_tensor("wqg", [D, NH * HD], FP32, kind="ExternalInput").ap()
    wk_d = nc.dram_tensor("wkg", [D, HD], FP32, kind="ExternalInput").ap()
    wv_d = nc.dram_tensor("wvg", [D, HD], FP32, kind="ExternalInput").ap()
    wo_d = nc.dram_tensor("wog", [NH * HD, D], FP32, kind="ExternalInput").ap()
    cosT_d = nc.dram_tensor("cosT", [HD, S], FP32, kind="ExternalInput").ap()
    sinrT_d = nc.dram_tensor("sinrotT", [HD, S], FP32, kind="ExternalInput").ap()
    mdiag_d = nc.dram_tensor("maskdiag", [NCH * P, SC], FP32, kind="ExternalInput").ap()
    ident_d = nc.dram_tensor("ident", [P, P], FP32, kind="ExternalInput").ap()
    y_d = nc.dram_tensor("y", [S, D], FP32, kind="ExternalOutput").ap()

    with tile.TileContext(nc) as tc, ExitStack() as ctx:
        # ---------- pools that live for the whole kernel ----------
        qkv = ctx.enter_context(tc.tile_pool(name="qkv", bufs=1))
        small = ctx.enter_context(tc.tile_pool(name="small", bufs=1))

        # resident Q^T per head, K^T, V tiles
        qt = [qkv.tile([P, S], FP32, tag=f"qt{h}", name=f"qt{h}") for h in range(NH)]
        kt = qkv.tile([P, S], FP32, tag="kt", name="kt")
        v_tiles = [qkv.tile([P, HD], FP32, tag=f"v{k}", name=f"v{k}") for k in range(NDT)]

        ones_sb = small.tile([P, P], FP32, tag="ones")
        nc.gpsimd.memset(ones_sb[:], 1.0)
        ident_sb = small.tile([P, P], FP32, tag="ident")
        nc.sync.dma_start(ident_sb[:], ident_d[:])
        mdiag_sb = [small.tile([P, SC], FP32, tag=f"md{r}", name=f"md{r}") for r in range(NCH)]
        for r in range(NCH):
            nc.sync.dma_start(mdiag_sb[r][:], mdiag_d[r * P:(r + 1) * P, :])

        # ================= phase 1: QKV projection + RoPE =================
        with tc.tile_pool(name="p1", bufs=1) as p1, \
             tc.tile_pool(name="xin", bufs=24) as xin, \
             tc.tile_pool(name="rtmp", bufs=4) as rtmp, \
             tc.tile_pool(name="ps1", bufs=2, space="PSUM") as ps1:

            # weights resident for phase 1
            wq_sb = [p1.tile([P, NH * HD], FP32, tag=f"wq{k}", name=f"wq{k}") for k in range(NDT)]
            wk_sb = [p1.tile([P, HD], FP32, tag=f"wk{k}", name=f"wk{k}") for k in range(NDT)]
            wv_sb = [p1.tile([P, HD], FP32, tag=f"wv{k}", name=f"wv{k}") for k in range(NDT)]
            for k in range(NDT):
                nc.sync.dma_start(wq_sb[k][:], wq_d[k * P:(k + 1) * P, :])
                nc.sync.dma_start(wk_sb[k][:], wk_d[k * P:(k + 1) * P, :])
                nc.sync.dma_start(wv_sb[k][:], wv_d[k * P:(k + 1) * P, :])
            cosT_sb = p1.tile([HD, S], FP32, tag="cosT")
            sinrT_sb = p1.tile([HD, S], FP32, tag="sinrT")
            nc.sync.dma_start(cosT_sb[:], cosT_d[:])
            nc.sync.dma_start(sinrT_sb[:], sinrT_d[:])

            vT = p1.tile([HD, S], FP32, tag="vT")

            for sc in range(NCH):
                s0 = sc * SC
                xts = []
                for k in range(NDT):
                    xt = xin.tile([P, SC], FP32, tag="x")
                    nc.sync.dma_start(xt[:], xT_d[k * P:(k + 1) * P, s0:s0 + SC])
                    xts.append(xt)

                # m = 0..3: q heads; 4: k; 5: v
                for m in range(NH + 2):
                    psum = ps1.tile([P, SC], FP32, tag="proj")
                    for k in range(NDT):
                        if m < NH:
                            lhsT = wq_sb[k][:, m * HD:(m + 1) * HD]
                        elif m == NH:
                            lhsT = wk_sb[k][:]
                        else:
                            lhsT = wv_sb[k][:]
                        nc.tensor.matmul(
                            psum[:], _mm_ap(lhsT), _mm_ap(xts[k][:]),
                            start=(k == 0), stop=(k == NDT - 1),
                        )
                    if m <= NH:
                        # RoPE: dst = psum*cosT + shift(psum)*sinrotT
                        dst = (qt[m] if m < NH else kt)[:, s0:s0 + SC]
                        t0 = rtmp.tile([P, SC], FP32, tag="t0")
                        t1 = rtmp.tile([P, SC], FP32, tag="t1")
                        nc.vector.tensor_mul(t0[:], psum[:], cosT_sb[:, s0:s0 + SC])
                        nc.vector.tensor_mul(
                            t1[0:64, :], psum[64:128, :], sinrT_sb[0:64, s0:s0 + SC])
                        nc.vector.tensor_mul(
                            t1[64:128, :], psum[0:64, :], sinrT_sb[64:128, s0:s0 + SC])
                        nc.vector.tensor_add(dst, t0[:], t1[:])
                    else:
                        nc.vector.tensor_copy(vT[:, s0:s0 + SC], psum[:])

            # transpose V^T [HD,S] -> V tiles [S_k=128, HD]
            with tc.tile_pool(name="pst", bufs=2, space="PSUM") as pst:
                for k in range(NDT):
                    ps_t = pst.tile([P, P], FP32, tag="vt")
                    nc.tensor.transpose(
                        ps_t[:], vT[:, k * P:(k + 1) * P], ident_sb[:])
                    nc.vector.tensor_copy(v_tiles[k][:], ps_t[:])

        # ================= phase 2: attention + output projection =========
        with tc.tile_pool(name="p2", bufs=1) as p2, \
             tc.tile_pool(name="pt", bufs=20) as ptp, \
             tc.tile_pool(name="nrm", bufs=3) as nrm, \
             tc.tile_pool(name="ps2", bufs=2, space="PSUM") as ps2, \
             tc.tile_pool(name="pss", bufs=2, space="PSUM") as pss:

            wo_sb = [p2.tile([P, D], FP32, tag=f"wo{h}", name=f"wo{h}") for h in range(NH)]
            for h in range(NH):
                nc.sync.dma_start(wo_sb[h][:], wo_d[h * P:(h + 1) * P, :])
            outT = [p2.tile([P, SC], FP32, tag=f"ot{h}", name=f"ot{h}") for h in range(NH)]

            for c in range(NCH):
                q0 = c * SC
                nk = 4 * c + 4          # active sk tiles (causal)
                for h in range(NH):
                    pts = []
                    # all-ones stationary -> every psum partition gets the
                    # column sum over sk (the broadcast we need, for free)
                    sums_ps = pss.tile([P, SC], FP32, tag="sums")
                    for k in range(nk):
                        sc_ps = ps2.tile([P, SC], FP32, tag="sc")
                        nc.tensor.matmul(
                            sc_ps[:],
                            _mm_ap(kt[:, k * P:(k + 1) * P]),
                            _mm_ap(qt[h][:, q0:q0 + SC]),
                            start=True, stop=True,
                        )
                        pt = ptp.tile([P, SC], FP32, tag="pt")
                        if k >= 4 * c:
                            # diagonal block: scores*scale + mask, then exp
                            r = k % NCH
                            nc.vector.scalar_tensor_tensor(
                                sc_ps[:], sc_ps[:], SCALE, mdiag_sb[r][:],
                                op0=mybir.AluOpType.mult, op1=mybir.AluOpType.add)
                            nc.scalar.activation(
                                pt[:], sc_ps[:], mybir.ActivationFunctionType.Exp)
                        else:
                            nc.scalar.activation(
                                pt[:], sc_ps[:], mybir.ActivationFunctionType.Exp,
                                scale=SCALE)
                        pts.append(pt)
                        # row-sums over sk via ones-matmul, accumulated in PSUM
                        nc.tensor.matmul(
                            sums_ps[:], _mm_ap(ones_sb[:]), _mm_ap(pt[:]),
                            start=(k == 0), stop=(k == nk - 1),
                        )
                    # AV: outT_h [HD, sq] = sum_k V_k^T-free @ probsT_k
                    av_ps = ps2.tile([P, SC], FP32, tag="av")
                    for k in range(nk):
                        nc.tensor.matmul(
                            av_ps[:], _mm_ap(v_tiles[k][:]), _mm_ap(pts[k][:]),
                            start=(k == 0), stop=(k == nk - 1),
                        )
                    # normalize: outT[h] = av * (1/sums); sums already
                    # replicated across partitions by the ones-matmul
                    recip = nrm.tile([P, SC], FP32, tag="recip")
                    nc.vector.reciprocal(recip[:], sums_ps[:])
                    nc.vector.tensor_mul(outT[h][:], av_ps[:], recip[:])

                # output projection for this sq chunk
                for t in range(SC // P):
                    row0 = q0 + t * P
                    for dci in range(NCH):
                        d0 = dci * SC
                        y_ps = ps2.tile([P, SC], FP32, tag="y")
                        for h in range(NH):
                            nc.tensor.matmul(
                                y_ps[:],
                                _mm_ap(outT[h][:, t * P:(t + 1) * P]),
                                _mm_ap(wo_sb[h][:, d0:d0 + SC]),
                                start=(h == 0), stop=(h == NH - 1),
                            )
                        y_sb = nrm.tile([P, SC], FP32, tag="ysb")
                        nc.vector.tensor_copy(y_sb[:], y_ps[:])
                        nc.sync.dma_start(y_d[row0:row0 + P, d0:d0 + SC], y_sb[:])

    nc.compile()
    return nc


def _host_tables():
    inv_freq = 1.0 / (THETA ** (np.arange(0, HD, 2, dtype=np.float32) / HD))
    t = np.arange(S, dtype=np.float32)
    freqs = t[:, None] * inv_freq[None, :]              # [S, HD/2]
    emb = np.concatenate([freqs, freqs], axis=-1)       # [S, HD]
    cos = np.cos(emb).astype(np.float32)
    sin = np.sin(emb).astype(np.float32)
    cosT = np.ascontiguousarray(cos.T)                  # [HD, S]
    sinT = np.ascontiguousarray(sin.T)
    sinrotT = sinT.copy()
    sinrotT[0:HD // 2] = -sinT[0:HD // 2]
    return cosT, sinrotT


def get_program():
    if "nc" not in _CACHE:
        _CACHE["nc"] = _build_program()
    return _CACHE["nc"]


def make_in_maps(x, wq, wk, wv, wo, mask):
    x = np.asarray(x, dtype=np.float32)
    wq = np.asarray(wq, dtype=np.float32)
    wk = np.asarray(wk, dtype=np.float32)
    wv = np.asarray(wv, dtype=np.float32)
    wo = np.asarray(wo, dtype=np.float32)
    mask = np.asarray(mask, dtype=np.float32)

    cosT, sinrotT = _host_tables()
    ident = np.eye(P, dtype=np.float32)
    # maskdiag[r*128+a, b] = mask[0,0, b, r*128+a]  (pattern repeats per chunk)
    maskdiag = np.ascontiguousarray(mask[0, 0, 0:SC, 0:SC].T)

    xT = [np.ascontiguousarray(x[b].T) for b in range(B)]
    in_maps = []
    for c in range(NCORES):
        b, g = c // NG, c % NG
        qc0 = g * NH * HD
        kc0 = g * HD
        in_maps.append({
            "xT": xT[b],
            "wqg": np.ascontiguousarray(wq[:, qc0:qc0 + NH * HD]),
            "wkg": np.ascontiguousarray(wk[:, kc0:kc0 + HD]),
            "wvg": np.ascontiguousarray(wv[:, kc0:kc0 + HD]),
            "wog": np.ascontiguousarray(wo[qc0:qc0 + NH * HD, :]),
            "cosT": cosT,
            "sinrotT": sinrotT,
            "maskdiag": maskdiag,
            "ident": ident,
        })
    return in_maps


LAST_RESULTS = None


def _make_exec(nc):
    """Mirror run_bass_via_pjrt's multi-core path, but keep the jitted
    executable so repeated (timed) dispatches skip retrace/reload."""
    import jax
    from jax.experimental.shard_map import shard_map
    from jax.sharding import Mesh, PartitionSpec

    from concourse import bass2jax, mybir as _mybir

    bass2jax.install_neuronx_cc_hook()
    partition_name = (
        nc.partition_id_tensor.name if nc.partition_id_tensor else None)
    in_names, out_names, out_avals, zero_outs = [], [], [], []
    for alloc in nc.m.functions[0].allocations:
        if not isinstance(alloc, _mybir.MemoryLocationSet):
            continue
        name = alloc.memorylocations[0].name
        if alloc.kind == "ExternalInput":
            if name != partition_name:
                in_names.append(name)
        elif alloc.kind == "ExternalOutput":
            shape = tuple(alloc.tensor_shape)
            dtype = _mybir.dt.np(alloc.dtype)
            out_names.append(name)
            out_avals.append(jax.core.ShapedArray(shape, dtype))
            zero_outs.append(np.zeros(shape, dtype))
    n_params = len(in_names)
    n_outs = len(out_avals)
    all_in_names = list(in_names) + list(out_names)
    if partition_name is not None:
        all_in_names.append(partition_name)
    donate = tuple(range(n_params, n_params + n_outs))

    def _body(*args):
        operands = list(args)
        if partition_name is not None:
            operands.append(bass2jax.partition_id_tensor())
        outs = bass2jax._bass_exec_p.bind(
            *operands,
            out_avals=tuple(out_avals),
            in_names=tuple(all_in_names),
            out_names=tuple(out_names),
            lowering_input_output_aliases=(),
            sim_require_finite=True,
            sim_require_nnan=True,
            nc=nc,
        )
        return tuple(outs)

    devices = jax.devices()[:NCORES]
    mesh = Mesh(np.asarray(devices), ("core",))
    sharded = jax.jit(
        shard_map(
            _body, mesh=mesh,
            in_specs=(PartitionSpec("core"),) * (n_params + n_outs),
            out_specs=(PartitionSpec("core"),) * n_outs,
            check_rep=False,
        ),
        donate_argnums=donate, keep_unused=True,
    )
    return {
        "fn": sharded, "in_names": in_names, "out_names": out_names,
        "out_avals": out_avals, "zero_outs": zero_outs, "mesh": mesh,
    }


def get_exec():
    if "exec" not in _CACHE:
        _CACHE["exec"] = _make_exec(get_program())
    return _CACHE["exec"]


def _concat_inputs(ex, in_maps):
    return [
        np.concatenate([np.asarray(in_maps[c][name]) for c in range(NCORES)], axis=0)
        for name in ex["in_names"]
    ]


def _concat_zeros(ex):
    return [
        np.zeros((NCORES * z.shape[0], *z.shape[1:]), z.dtype)
        for z in ex["zero_outs"]
    ]


def run_on_device(in_maps):
    """One dispatch; returns per-core output dicts (numpy)."""
    ex = get_exec()
    out_arrs = ex["fn"](*_concat_inputs(ex, in_maps), *_concat_zeros(ex))
    res = []
    for c in range(NCORES):
        res.append({
            name: np.asarray(out_arrs[i]).reshape(
                NCORES, *ex["out_avals"][i].shape)[c]
            for i, name in enumerate(ex["out_names"])
        })
    return res


def bench(in_maps, iters=5):
    """Timed repeated dispatch: inputs pre-placed on device, fresh donated
    zero output buffers pre-placed per iteration. Returns list of wall ns."""
    import time

    import jax
    from jax.sharding import NamedSharding, PartitionSpec

    ex = get_exec()
    sh = NamedSharding(ex["mesh"], PartitionSpec("core"))
    dev_in = [jax.device_put(a, sh) for a in _concat_inputs(ex, in_maps)]
    zsets = [[jax.device_put(z, sh) for z in _concat_zeros(ex)]
             for _ in range(iters + 1)]
    jax.block_until_ready(dev_in)
    jax.block_until_ready(zsets)
    out = ex["fn"](*dev_in, *zsets[0])       # warm-up
    jax.block_until_ready(out)
    times = []
    for i in range(iters):
        t0 = time.perf_counter()
        out = ex["fn"](*dev_in, *zsets[i + 1])
        jax.block_until_ready(out)
        times.append((time.perf_counter() - t0) * 1e9)
    return times


def kernel(x, wq, wk, wv, wo, mask):
    global LAST_RESULTS
    in_maps = make_in_maps(x, wq, wk, wv, wo, mask)
    results = run_on_device(in_maps)
    LAST_RESULTS = results
    out = np.zeros((B, S, D), dtype=np.float32)
    for c in range(NCORES):
        b = c // NG
        out[b] += results[c]["y"]
    return out


# revision 17
# speedup vs baseline: 68.4760x; 68.4760x over previous
"""Trainium2 Bass kernel for GQA attention (B=2,S=2048,D=2048,H=16,KV=4,HD=128)
with RoPE + causal mask, sharded over 8 NeuronCores:
  2-way data parallel over batch x 4-way tensor parallel over KV groups.

Core c = (b, g): b = c // 4, g = c % 4.
Each core computes, for its batch b and KV group g (q heads 4g..4g+3):
  QT_h [HD,S], KT [HD,S] (RoPE'd), V [S,HD]    via matmul vs xT [D,S]
  scoresT [sk,sq] blocks, exp on ScalarE (scale folded), row-sums via an
  all-ones matmul (which also replicates the sums across partitions),
  AV with V tiles stationary -> outT [HD,sq], per-head normalization via
  reciprocal, partial y = attn_norm @ wo_rows[g]; host sums the 4 partials.

matmul(out, lhsT, rhs) = lhsT.T @ rhs, contraction over the partition dim.
All contractions are K=128.  Causality at block granularity: fully-masked
(sk,sq) blocks skipped; diagonal blocks add the mask slice (pattern repeats
every 4 sk-tiles, so only a [512,512] mask transpose is shipped).
"""

import os
from contextlib import ExitStack

import numpy as np

import concourse.bacc as bacc
import concourse.mybir as mybir
import concourse.tile as tile

# ---------------- problem constants (hardcoded per contract) ----------------
B, S, D = 2, 2048, 2048
H, KV, HD = 16, 4, 128
REP = H // KV            # 4 q heads per kv head
NG = KV                  # 4 tensor-parallel groups
NCORES = 8
THETA = 10000.0
SCALE = 1.0 / float(np.sqrt(HD))

P = 128                  # partition dim
SC = 512                 # moving free-dim chunk (fp32 max)
NDT = S // P             # 16 tiles of 128 along S or D
NCH = S // SC            # 4 chunks of 512 along S
NH = REP                 # 4 q-heads per core

FP32 = mybir.dt.float32
F32R = mybir.dt.float32r

# matmul dtype: "fp32" (exact, 4 cyc/row) or "f32r" (tf32-class, 1 cyc/row)
MM_MODE = os.environ.get("KERNEL_MM_MODE", "fp32")

_CACHE = {}


def _build_program(mm_mode=MM_MODE, repeat=1):
    # MDT: dtype of every matmul operand tile (and the DRAM tensors DMA'd
    # straight into them -- the BIR verifier requires fp32r matmult inputs
    # to be *produced* as fp32r).
    MDT = F32R if mm_mode == "f32r" else FP32

    nc = bacc.Bacc("TRN2", target_bir_lowering=False, debug=False)

    xT_d = nc.dram_tensor("xT", [D, S], MDT, kind="ExternalInput").ap()
    wq_d = nc.dram_tensor("wqg", [D, NH * HD], MDT, kind="ExternalInput").ap()
    wk_d = nc.dram_tensor("wkg", [D, HD], MDT, kind="ExternalInput").ap()
    wv_d = nc.dram_tensor("wvg", [D, HD], MDT, kind="ExternalInput").ap()
    wo_d = nc.dram_tensor("wog", [NH * HD, D], MDT, kind="ExternalInput").ap()
    cosT_d = nc.dram_tensor("cosT", [HD, S], FP32, kind="ExternalInput").ap()
    sinrT_d = nc.dram_tensor("sinrotT", [HD, S], FP32, kind="ExternalInput").ap()
    mdiag_d = nc.dram_tensor("maskdiag", [NCH * P, SC], FP32, kind="ExternalInput").ap()
    ident_d = nc.dram_tensor("ident", [P, P], FP32, kind="ExternalInput").ap()
    y_d = nc.dram_tensor("y", [S, D], FP32, kind="ExternalOutput").ap()

    with tile.TileContext(nc) as tc, ExitStack() as ctx:
        qkv = ctx.enter_context(tc.tile_pool(name="qkv", bufs=1))
        small = ctx.enter_context(tc.tile_pool(name="small", bufs=1))

        # resident Q^T per head, K^T, V tiles
        qt = [qkv.tile([P, S], MDT, tag=f"qt{h}", name=f"qt{h}") for h in range(NH)]
        kt = qkv.tile([P, S], MDT, tag="kt", name="kt")
        v_tiles = [qkv.tile([P, HD], MDT, tag=f"v{k}", name=f"v{k}")
                   for k in range(NDT)]

        ones_sb = small.tile([P, P], MDT, tag="ones")
        nc.gpsimd.memset(ones_sb[:], 1.0)
        ident_sb = small.tile([P, P], FP32, tag="ident")
        nc.sync.dma_start(ident_sb[:], ident_d[:])

        for rep in range(repeat):
            # ============== phase 1: QKV projection + RoPE ==============
            with tc.tile_pool(name="p1", bufs=1) as p1, \
                 tc.tile_pool(name="xin", bufs=7) as xin, \
                 tc.tile_pool(name="rtmp", bufs=2) as rtmp, \
                 tc.tile_pool(name="ps1", bufs=2, space="PSUM") as ps1:

                # weight slab: tile k of wq lives at slab columns
                # [k*512, (k+1)*512), head slice m at [k*512 + m*128, ...).
                # Loads are split into quarters and spread over two DMA
                # queues (sync + scalar) so the first matmul chain is gated
                # by ~2MB, not the full 13MB of phase-1 inputs.
                XQ = NDT // 4   # 4 d-tiles per quarter slab
                wqs = p1.tile([P, NDT * NH * HD], MDT, tag="wqs")
                wks = p1.tile([P, NDT * HD], MDT, tag="wks")
                wvs = p1.tile([P, NDT * HD], MDT, tag="wvs")
                cosT_sb = p1.tile([HD, S], FP32, tag="cosT")
                sinrT_sb = p1.tile([HD, S], FP32, tag="sinrT")
                for qq in range(4):
                    r0, r1 = qq * XQ * P, (qq + 1) * XQ * P
                    nc.scalar.dma_start(
                        wqs[:, qq * XQ * NH * HD:(qq + 1) * XQ * NH * HD]
                        .rearrange("p (n m) -> p n m", n=XQ),
                        wq_d[r0:r1, :].rearrange("(n p) m -> p n m", p=P))
                nc.gpsimd.dma_start(cosT_sb[:], cosT_d[:])
                nc.gpsimd.dma_start(sinrT_sb[:], sinrT_d[:])
                nc.gpsimd.dma_start(
                    wks[:].rearrange("p (n m) -> p n m", n=NDT),
                    wk_d.rearrange("(n p) m -> p n m", p=P))
                nc.gpsimd.dma_start(
                    wvs[:].rearrange("p (n m) -> p n m", n=NDT),
                    wv_d.rearrange("(n p) m -> p n m", p=P))

                vT = p1.tile([HD, S], FP32, tag="vT")

                for sc in range(NCH):
                    s0 = sc * SC
                    xq_slabs = []
                    for qq in range(4):
                        xs = xin.tile([P, XQ * SC], MDT, tag="x")
                        eng = nc.sync if qq % 2 == 0 else nc.scalar
                        eng.dma_start(
                            xs[:].rearrange("p (n s) -> p n s", n=XQ),
                            xT_d[qq * XQ * P:(qq + 1) * XQ * P, s0:s0 + SC]
                            .rearrange("(n p) s -> p n s", p=P))
                        xq_slabs.append(xs)

                    def xts_k(k):
                        return xq_slabs[k // XQ][:, (k % XQ) * SC:(k % XQ + 1) * SC]

                    # m = 0..3: q heads; 4: k; 5: v
                    for m in range(NH + 2):
                        psum = ps1.tile([P, SC], FP32, tag="proj")
                        for k in range(NDT):
                            if m < NH:
                                lhsT = wqs[:, k * NH * HD + m * HD:
                                           k * NH * HD + (m + 1) * HD]
                            elif m == NH:
                                lhsT = wks[:, k * HD:(k + 1) * HD]
                            else:
                                lhsT = wvs[:, k * HD:(k + 1) * HD]
                            nc.tensor.matmul(
                                psum[:], lhsT, xts_k(k),
                                start=(k == 0), stop=(k == NDT - 1),
                            )
                        if m <= NH:
                            # RoPE: dst = q*cosT + shift(q)*sinrotT.
                            # ScalarE stages psum->SBUF (idle in phase 1);
                            # DVE then runs SBUF-only in its 2x fp32 mode.
                            dst = (qt[m] if m < NH else kt)[:, s0:s0 + SC]
                            q_sb = rtmp.tile([P, SC], FP32, tag="q")
                            t1 = rtmp.tile([P, SC], FP32, tag="t1")
                            nc.scalar.copy(q_sb[:], psum[:])
                            nc.vector.tensor_mul(
                                t1[0:64, :], q_sb[64:128, :],
                                sinrT_sb[0:64, s0:s0 + SC])
                            nc.vector.tensor_mul(
                                t1[64:128, :], q_sb[0:64, :],
                                sinrT_sb[64:128, s0:s0 + SC])
                            nc.vector.tensor_mul(
                                q_sb[:], q_sb[:], cosT_sb[:, s0:s0 + SC])
                            nc.vector.tensor_add(dst, q_sb[:], t1[:])
                        else:
                            nc.scalar.copy(vT[:, s0:s0 + SC], psum[:])

                # transpose V^T [HD,S] -> V tiles [S_k=128, HD]
                with tc.tile_pool(name="pst", bufs=2, space="PSUM") as pst:
                    for k in range(NDT):
                        ps_t = pst.tile([P, P], FP32, tag="vt")
                        nc.tensor.transpose(
                            ps_t[:], vT[:, k * P:(k + 1) * P], ident_sb[:])
                        nc.scalar.copy(v_tiles[k][:], ps_t[:])

            # ========== phase 2: attention + output projection ==========
            with tc.tile_pool(name="p2", bufs=1) as p2, \
                 tc.tile_pool(name="pt", bufs=20) as ptp, \
                 tc.tile_pool(name="nrm", bufs=3) as nrm, \
                 tc.tile_pool(name="yst", bufs=2) as yst, \
                 tc.tile_pool(name="ps2", bufs=2, space="PSUM") as ps2, \
                 tc.tile_pool(name="pss", bufs=2, space="PSUM") as pss:

                wos = p2.tile([P, NH * D], MDT, tag="wos")
                nc.sync.dma_start(
                    wos[:].rearrange("p (n d) -> p n d", n=NH),
                    wo_d.rearrange("(n p) d -> p n d", p=P))
                wo_sb = [wos[:, h * D:(h + 1) * D] for h in range(NH)]
                mdiag_slab = p2.tile([P, NCH * SC], FP32, tag="mds")
                nc.gpsimd.dma_start(
                    mdiag_slab[:].rearrange("p (r s) -> p r s", r=NCH),
                    mdiag_d.rearrange("(r p) s -> p r s", p=P))
                mdiag_sb = [mdiag_slab[:, r * SC:(r + 1) * SC]
                            for r in range(NCH)]
                outT = [p2.tile([P, SC], MDT, tag=f"ot{h}", name=f"ot{h}")
                        for h in range(NH)]

                for c in range(NCH):
                    q0 = c * SC
                    nk = 4 * c + 4          # active sk tiles (causal)
                    for h in range(NH):
                        pts = []
                        offs = []
                        # all-ones stationary -> every psum partition gets
                        # the column sum over sk (broadcast for free)
                        sums_ps = pss.tile([P, SC], FP32, tag="sums")
                        for k in range(nk):
                            # diagonal blocks: sk tile k only attends to
                            # sq >= 128k, i.e. chunk columns [off:512).
                            # f32r matmuls need moving dim >= 256 for the
                            # 1 cyc/row mode, so keep at least 256 columns
                            # (the extra columns are masked -> exp -> 0).
                            off = max(0, (k - 4 * c) * P)
                            if MDT == F32R:
                                off = min(off, SC - 2 * P)
                            sc_ps = ps2.tile([P, SC], FP32, tag="sc")
                            nc.tensor.matmul(
                                sc_ps[:, off:],
                                kt[:, k * P:(k + 1) * P],
                                qt[h][:, q0 + off:q0 + SC],
                                start=True, stop=True,
                            )
                            pt = ptp.tile([P, SC], MDT, tag="pt")
                            if k >= 4 * c:
                                # diagonal block: scores*scale + mask, exp
                                r = k % NCH
                                nc.vector.scalar_tensor_tensor(
                                    sc_ps[:, off:], sc_ps[:, off:], SCALE,
                                    mdiag_sb[r][:, off:],
                                    op0=mybir.AluOpType.mult,
                                    op1=mybir.AluOpType.add)
                                nc.scalar.activation(
                                    pt[:, off:], sc_ps[:, off:],
                                    mybir.ActivationFunctionType.Exp)
                            else:
                                nc.scalar.activation(
                                    pt[:, off:], sc_ps[:, off:],
                                    mybir.ActivationFunctionType.Exp,
                                    scale=SCALE)
                            pts.append(pt)
                            offs.append(off)
                            nc.tensor.matmul(
                                sums_ps[:, off:], ones_sb[:], pt[:, off:],
                                start=(k == 0), stop=(k == nk - 1),
                            )
                        # AV: outT_h [HD, sq] = sum_k V_k^T @ probsT_k
                        av_ps = ps2.tile([P, SC], FP32, tag="av")
                        for k in range(nk):
                            nc.tensor.matmul(
                                av_ps[:, offs[k]:], v_tiles[k][:],
                                pts[k][:, offs[k]:],
                                start=(k == 0), stop=(k == nk - 1),
                            )
                        # normalize: outT[h] = av * (1/sums)
                        recip = nrm.tile([P, SC], FP32, tag="recip")
                        nc.vector.reciprocal(recip[:], sums_ps[:])
                        nc.vector.tensor_mul(outT[h][:], av_ps[:], recip[:])

                    # output projection for this sq chunk; results are
                    # staged in half-slabs (t pairs) and stored with one
                    # batched DMA each on the otherwise-idle gpsimd queue
                    for half in range(2):
                        yslab = yst.tile([P, 2 * D], FP32, tag="yslab")
                        for tt in range(2):
                            t = half * 2 + tt
                            for dci in range(NCH):
                                d0 = dci * SC
                                y_ps = ps2.tile([P, SC], FP32, tag="y")
                                for h in range(NH):
                                    nc.tensor.matmul(
                                        y_ps[:],
                                        outT[h][:, t * P:(t + 1) * P],
                                        wo_sb[h][:, d0:d0 + SC],
                                        start=(h == 0), stop=(h == NH - 1),
                                    )
                                nc.vector.tensor_copy(
                                    yslab[:, tt * D + d0:tt * D + d0 + SC],
                                    y_ps[:])
                        row0 = q0 + half * 2 * P
                        nc.gpsimd.dma_start(
                            y_d[row0:row0 + 2 * P, :]
                            .rearrange("(t p) d -> p t d", p=P),
                            yslab[:].rearrange("p (t d) -> p t d", t=2))

    nc.compile()
    return nc


def _host_tables():
    inv_freq = 1.0 / (THETA ** (np.arange(0, HD, 2, dtype=np.float32) / HD))
    t = np.arange(S, dtype=np.float32)
    freqs = t[:, None] * inv_freq[None, :]              # [S, HD/2]
    emb = np.concatenate([freqs, freqs], axis=-1)       # [S, HD]
    cos = np.cos(emb).astype(np.float32)
    sin = np.sin(emb).astype(np.float32)
    cosT = np.ascontiguousarray(cos.T)                  # [HD, S]
    sinT = np.ascontiguousarray(sin.T)
    sinrotT = sinT.copy()
    sinrotT[0:HD // 2] = -sinT[0:HD // 2]
    return cosT, sinrotT


def get_program(mm_mode=MM_MODE, repeat=1):
    key = ("nc", mm_mode, repeat)
    if key not in _CACHE:
        _CACHE[key] = _build_program(mm_mode, repeat)
    return _CACHE[key]


def make_in_maps(x, wq, wk, wv, wo, mask):
    x = np.asarray(x, dtype=np.float32)
    wq = np.asarray(wq, dtype=np.float32)
    wk = np.asarray(wk, dtype=np.float32)
    wv = np.asarray(wv, dtype=np.float32)
    wo = np.asarray(wo, dtype=np.float32)
    mask = np.asarray(mask, dtype=np.float32)

    cosT, sinrotT = _host_tables()
    ident = np.eye(P, dtype=np.float32)
    # maskdiag[r*128+a, b] = mask[0,0, b, r*128+a]; pattern repeats per chunk
    maskdiag = np.ascontiguousarray(mask[0, 0, 0:SC, 0:SC].T)

    xT = [np.ascontiguousarray(x[b].T) for b in range(B)]
    in_maps = []
    for c in range(NCORES):
        b, g = c // NG, c % NG
        qc0 = g * NH * HD
        kc0 = g * HD
        in_maps.append({
            "xT": xT[b],
            "wqg": np.ascontiguousarray(wq[:, qc0:qc0 + NH * HD]),
            "wkg": np.ascontiguousarray(wk[:, kc0:kc0 + HD]),
            "wvg": np.ascontiguousarray(wv[:, kc0:kc0 + HD]),
            "wog": np.ascontiguousarray(wo[qc0:qc0 + NH * HD, :]),
            "cosT": cosT,
            "sinrotT": sinrotT,
            "maskdiag": maskdiag,
            "ident": ident,
        })
    return in_maps


LAST_RESULTS = None


def _make_exec(nc):
    """Mirror run_bass_via_pjrt's multi-core path, but keep the jitted
    executable so repeated (timed) dispatches skip retrace/reload."""
    import jax
    from jax.experimental.shard_map import shard_map
    from jax.sharding import Mesh, PartitionSpec

    from concourse import bass2jax, mybir as _mybir

    bass2jax.install_neuronx_cc_hook()
    partition_name = (
        nc.partition_id_tensor.name if nc.partition_id_tensor else None)
    in_names, out_names, out_avals, zero_outs = [], [], [], []
    for alloc in nc.m.functions[0].allocations:
        if not isinstance(alloc, _mybir.MemoryLocationSet):
            continue
        name = alloc.memorylocations[0].name
        if alloc.kind == "ExternalInput":
            if name != partition_name:
                in_names.append(name)
        elif alloc.kind == "ExternalOutput":
            shape = tuple(alloc.tensor_shape)
            dtype = _mybir.dt.np(alloc.dtype)
            out_names.append(name)
            out_avals.append(jax.core.ShapedArray(shape, dtype))
            zero_outs.append(np.zeros(shape, dtype))
    n_params = len(in_names)
    n_outs = len(out_avals)
    all_in_names = list(in_names) + list(out_names)
    if partition_name is not None:
        all_in_names.append(partition_name)
    donate = tuple(range(n_params, n_params + n_outs))

    def _body(*args):
        operands = list(args)
        if partition_name is not None:
            operands.append(bass2jax.partition_id_tensor())
        outs = bass2jax._bass_exec_p.bind(
            *operands,
            out_avals=tuple(out_avals),
            in_names=tuple(all_in_names),
            out_names=tuple(out_names),
            lowering_input_output_aliases=(),
            sim_require_finite=True,
            sim_require_nnan=True,
            nc=nc,
        )
        return tuple(outs)

    devices = jax.devices()[:NCORES]
    mesh = Mesh(np.asarray(devices), ("core",))
    sharded = jax.jit(
        shard_map(
            _body, mesh=mesh,
            in_specs=(PartitionSpec("core"),) * (n_params + n_outs),
            out_specs=(PartitionSpec("core"),) * n_outs,
            check_rep=False,
        ),
        donate_argnums=donate, keep_unused=True,
    )
    return {
        "fn": sharded, "in_names": in_names, "out_names": out_names,
        "out_avals": out_avals, "zero_outs": zero_outs, "mesh": mesh,
    }


def get_exec(mm_mode=MM_MODE, repeat=1):
    key = ("exec", mm_mode, repeat)
    if key not in _CACHE:
        _CACHE[key] = _make_exec(get_program(mm_mode, repeat))
    return _CACHE[key]


def _concat_inputs(ex, in_maps):
    return [
        np.concatenate([np.asarray(in_maps[c][name]) for c in range(NCORES)],
                       axis=0)
        for name in ex["in_names"]
    ]


def _concat_zeros(ex):
    return [
        np.zeros((NCORES * z.shape[0], *z.shape[1:]), z.dtype)
        for z in ex["zero_outs"]
    ]


def run_on_device(in_maps, mm_mode=MM_MODE, repeat=1):
    """One dispatch; returns per-core output dicts (numpy)."""
    ex = get_exec(mm_mode, repeat)
    out_arrs = ex["fn"](*_concat_inputs(ex, in_maps), *_concat_zeros(ex))
    res = []
    for c in range(NCORES):
        res.append({
            name: np.asarray(out_arrs[i]).reshape(
                NCORES, *ex["out_avals"][i].shape)[c]
            for i, name in enumerate(ex["out_names"])
        })
    return res


def bench(in_maps, iters=5, mm_mode=MM_MODE, repeat=1):
    """Timed repeated dispatch: inputs pre-placed on device, fresh donated
    zero output buffers pre-placed per iteration. Returns list of wall ns."""
    import time

    import jax
    from jax.sharding import NamedSharding, PartitionSpec

    ex = get_exec(mm_mode, repeat)
    sh = NamedSharding(ex["mesh"], PartitionSpec("core"))
    dev_in = [jax.device_put(a, sh) for a in _concat_inputs(ex, in_maps)]
    zsets = [[jax.device_put(z, sh) for z in _concat_zeros(ex)]
             for _ in range(iters + 1)]
    jax.block_until_ready(dev_in)
    jax.block_until_ready(zsets)
    out = ex["fn"](*dev_in, *zsets[0])       # warm-up
    jax.block_until_ready(out)
    times = []
    for i in range(iters):
        t0 = time.perf_counter()
        out = ex["fn"](*dev_in, *zsets[i + 1])
        jax.block_until_ready(out)
        times.append((time.perf_counter() - t0) * 1e9)
    return times


def bench_slope(in_maps, iters=8, mm_mode=MM_MODE, r_hi=4):
    """Per-iteration kernel time via slope: (T(r_hi) - T(1)) / (r_hi - 1).
    Immune to constant dispatch overhead."""
    t1 = bench(in_maps, iters=iters, mm_mode=mm_mode, repeat=1)
    th = bench(in_maps, iters=iters, mm_mode=mm_mode, repeat=r_hi)
    t1m, thm = np.median(t1), np.median(th)
    t1b, thb = min(t1), min(th)
    return {
        "t1": t1, "th": th,
        "exec_ns_median": (thm - t1m) / (r_hi - 1),
        "exec_ns_min": (thb - t1b) / (r_hi - 1),
    }


def kernel(x, wq, wk, wv, wo, mask):
    global LAST_RESULTS
    in_maps = make_in_maps(x, wq, wk, wv, wo, mask)
    results = run_on_device(in_maps)
    LAST_RESULTS = results
    out = np.zeros((B, S, D), dtype=np.float32)
    for c in range(NCORES):
        b = c // NG
        out[b] += results[c]["y"]
    return out


# revision 31
# speedup vs baseline: 23912.4150x; 349.2088x over previous
"""Trainium2 Bass kernel for GQA attention (B=2,S=2048,D=2048,H=16,KV=4,HD=128)
with RoPE + causal mask, sharded over 8 NeuronCores:
  2-way data parallel over batch x 4-way tensor parallel over KV groups.

Core c = (b, g): b = c // 4, g = c % 4.
Each core computes, for its batch b and KV group g (q heads 4g..4g+3):
  QT_h [HD,S], KT [HD,S] (RoPE'd), V [S,HD]    via matmul vs xT [D,S]
  scoresT [sk,sq] blocks, exp on ScalarE (scale folded), row-sums via an
  all-ones matmul (which also replicates the sums across partitions),
  AV with V tiles stationary -> outT [HD,sq], per-head normalization via
  reciprocal, partial y = attn_norm @ wo_rows[g]; host sums the 4 partials.

matmul(out, lhsT, rhs) = lhsT.T @ rhs, contraction over the partition dim.
All contractions are K=128.  Causality at block granularity: fully-masked
(sk,sq) blocks skipped; diagonal blocks add the mask slice (pattern repeats
every 4 sk-tiles, so only a [512,512] mask transpose is shipped).
"""

import os
from contextlib import ExitStack

import numpy as np

import concourse.bacc as bacc
import concourse.mybir as mybir
import concourse.tile as tile

# ---------------- problem constants (hardcoded per contract) ----------------
B, S, D = 2, 2048, 2048
H, KV, HD = 16, 4, 128
REP = H // KV            # 4 q heads per kv head
NG = KV                  # 4 tensor-parallel groups
NCORES = 8
THETA = 10000.0
SCALE = 1.0 / float(np.sqrt(HD))

P = 128                  # partition dim
SC = 512                 # moving free-dim chunk (fp32 max)
NDT = S // P             # 16 tiles of 128 along S or D
NCH = S // SC            # 4 chunks of 512 along S
NH = REP                 # 4 q-heads per core

FP32 = mybir.dt.float32
F32R = mybir.dt.float32r

# matmul dtype: "f32r" (tf32-class operands, 1 cyc/row, measured full-kernel
# relative error ~2.2e-4 vs the fp32 reference) or "fp32" (exact to ~1.4e-6,
# but 4 cyc/row on the PE).  The attention output is a softmax-weighted
# average followed by modest-depth sums, so tf32-class operand rounding
# stays well inside a scale-relative tolerance.
MM_MODE = os.environ.get("KERNEL_MM_MODE", "f32r")

_CACHE = {}


def _build_program(mm_mode=MM_MODE, repeat=1):
    # MDT: dtype of every matmul operand tile (and the DRAM tensors DMA'd
    # straight into them -- the BIR verifier requires fp32r matmult inputs
    # to be *produced* as fp32r).
    MDT = F32R if mm_mode == "f32r" else FP32

    nc = bacc.Bacc("TRN2", target_bir_lowering=False, debug=False)

    xT_d = nc.dram_tensor("xT", [D, S], MDT, kind="ExternalInput").ap()
    wq_d = nc.dram_tensor("wqg", [D, NH * HD], MDT, kind="ExternalInput").ap()
    wk_d = nc.dram_tensor("wkg", [D, HD], MDT, kind="ExternalInput").ap()
    wv_d = nc.dram_tensor("wvg", [D, HD], MDT, kind="ExternalInput").ap()
    wo_d = nc.dram_tensor("wog", [NH * HD, D], MDT, kind="ExternalInput").ap()
    cosT_d = nc.dram_tensor("cosT", [HD, S], FP32, kind="ExternalInput").ap()
    sinrT_d = nc.dram_tensor("sinrotT", [HD, S], FP32, kind="ExternalInput").ap()
    mdiag_d = nc.dram_tensor("maskdiag", [NCH * P, SC], FP32, kind="ExternalInput").ap()
    ident_d = nc.dram_tensor("ident", [P, P], FP32, kind="ExternalInput").ap()
    ones_d = nc.dram_tensor("ones", [P, P], MDT, kind="ExternalInput").ap()
    y_d = nc.dram_tensor("y", [S, D], FP32, kind="ExternalOutput").ap()

    with tile.TileContext(nc) as tc, ExitStack() as ctx:
        qkv = ctx.enter_context(tc.tile_pool(name="qkv", bufs=1))
        small = ctx.enter_context(tc.tile_pool(name="small", bufs=1))

        # resident Q^T per head, K^T, V tiles
        qt = [qkv.tile([P, S], MDT, tag=f"qt{h}", name=f"qt{h}") for h in range(NH)]
        kt = qkv.tile([P, S], MDT, tag="kt", name="kt")
        v_tiles = [qkv.tile([P, HD], MDT, tag=f"v{k}", name=f"v{k}")
                   for k in range(NDT)]

        ones_sb = small.tile([P, P], MDT, tag="ones")
        ident_sb = small.tile([P, P], FP32, tag="ident")
        mdiag_slab = small.tile([P, NCH * SC], FP32, tag="mds")
        mdiag_sb = [mdiag_slab[:, r * SC:(r + 1) * SC] for r in range(NCH)]

        def load_consts():
            nc.gpsimd.dma_start(ones_sb[:], ones_d[:])
            nc.sync.dma_start(ident_sb[:], ident_d[:])
            nc.gpsimd.dma_start(
                mdiag_slab[:].rearrange("p (r s) -> p r s", r=NCH),
                mdiag_d.rearrange("(r p) s -> p r s", p=P))

        for rep in range(repeat):
            # ============== phase 1: QKV projection + RoPE ==============
            with tc.tile_pool(name="p1", bufs=1) as p1, \
                 tc.tile_pool(name="xin", bufs=6) as xin, \
                 tc.tile_pool(name="rtmp", bufs=2) as rtmp, \
                 tc.tile_pool(name="ps1", bufs=2, space="PSUM") as ps1:

                # weight slab: tile k of wq lives at slab columns
                # [k*512, (k+1)*512), head slice m at [k*512 + m*128, ...).
                # Loads are split into quarters and spread over two DMA
                # queues (sync + scalar) so the first matmul chain is gated
                # by ~2MB, not the full 13MB of phase-1 inputs.
                XQ = NDT // 4   # 4 d-tiles per quarter slab
                wqs = p1.tile([P, NDT * NH * HD], MDT, tag="wqs")
                wks = p1.tile([P, NDT * HD], MDT, tag="wks")
                wvs = p1.tile([P, NDT * HD], MDT, tag="wvs")
                cosT_sb = p1.tile([HD, S], FP32, tag="cosT")
                sinrT_sb = p1.tile([HD, S], FP32, tag="sinrT")
                for qq in range(4):
                    r0, r1 = qq * XQ * P, (qq + 1) * XQ * P
                    nc.scalar.dma_start(
                        wqs[:, qq * XQ * NH * HD:(qq + 1) * XQ * NH * HD]
                        .rearrange("p (n m) -> p n m", n=XQ),
                        wq_d[r0:r1, :].rearrange("(n p) m -> p n m", p=P))
                vT = p1.tile([HD, S], FP32, tag="vT")

                for sc in range(NCH):
                    s0 = sc * SC
                    xq_slabs = []
                    for qq in range(4):
                        xs = xin.tile([P, XQ * SC], MDT, tag="x")
                        nc.sync.dma_start(
                            xs[:].rearrange("p (n s) -> p n s", n=XQ),
                            xT_d[qq * XQ * P:(qq + 1) * XQ * P, s0:s0 + SC]
                            .rearrange("(n p) s -> p n s", p=P))
                        xq_slabs.append(xs)
                    if sc == 0:
                        # needed only from the first RoPE / m=4 onwards;
                        # issue after chunk-0 x so the first chain starts asap
                        nc.gpsimd.dma_start(cosT_sb[:], cosT_d[:])
                        nc.gpsimd.dma_start(sinrT_sb[:], sinrT_d[:])
                        nc.gpsimd.dma_start(
                            wks[:].rearrange("p (n m) -> p n m", n=NDT),
                            wk_d.rearrange("(n p) m -> p n m", p=P))
                        nc.gpsimd.dma_start(
                            wvs[:].rearrange("p (n m) -> p n m", n=NDT),
                            wv_d.rearrange("(n p) m -> p n m", p=P))
                        if rep == 0:
                            load_consts()

                    def xts_k(k):
                        return xq_slabs[k // XQ][:, (k % XQ) * SC:(k % XQ + 1) * SC]

                    # m = 0..3: q heads; 4: k; 5: v
                    for m in range(NH + 2):
                        psum = ps1.tile([P, SC], FP32, tag="proj", bufs=3)
                        for k in range(NDT):
                            if m < NH:
                                lhsT = wqs[:, k * NH * HD + m * HD:
                                           k * NH * HD + (m + 1) * HD]
                            elif m == NH:
                                lhsT = wks[:, k * HD:(k + 1) * HD]
                            else:
                                lhsT = wvs[:, k * HD:(k + 1) * HD]
                            nc.tensor.matmul(
                                psum[:], lhsT, xts_k(k),
                                start=(k == 0), stop=(k == NDT - 1),
                            )
                        if m <= NH:
                            # RoPE: dst = psum*cosT + shift(psum)*sinrotT
                            dst = (qt[m] if m < NH else kt)[:, s0:s0 + SC]
                            t0 = rtmp.tile([P, SC], FP32, tag="t0")
                            t1 = rtmp.tile([P, SC], FP32, tag="t1")
                            nc.vector.tensor_mul(
                                t0[:], psum[:], cosT_sb[:, s0:s0 + SC])
                            nc.vector.tensor_mul(
                                t1[0:64, :], psum[64:128, :],
                                sinrT_sb[0:64, s0:s0 + SC])
                            nc.vector.tensor_mul(
                                t1[64:128, :], psum[0:64, :],
                                sinrT_sb[64:128, s0:s0 + SC])
                            nc.vector.tensor_add(dst, t0[:], t1[:])
                        else:
                            nc.vector.tensor_copy(vT[:, s0:s0 + SC], psum[:])

                    # transpose this chunk of V^T -> V tiles [S_k=128, HD]
                    for kk in range(SC // P):
                        k = sc * (SC // P) + kk
                        ps_t = ps1.tile([P, P], FP32, tag="vt")
                        nc.tensor.transpose(
                            ps_t[:], vT[:, k * P:(k + 1) * P], ident_sb[:])
                        nc.vector.tensor_copy(v_tiles[k][:], ps_t[:])

            # ========== phase 2: attention + output projection ==========
            with tc.tile_pool(name="p2", bufs=1) as p2, \
                 tc.tile_pool(name="pt", bufs=20) as ptp, \
                 tc.tile_pool(name="nrm", bufs=3) as nrm, \
                 tc.tile_pool(name="yst", bufs=3) as yst, \
                 tc.tile_pool(name="ps2", bufs=2, space="PSUM") as ps2, \
                 tc.tile_pool(name="pss", bufs=2, space="PSUM") as pss:

                wos = p2.tile([P, NH * D], MDT, tag="wos")
                nc.sync.dma_start(
                    wos[:].rearrange("p (n d) -> p n d", n=NH),
                    wo_d.rearrange("(n p) d -> p n d", p=P))
                wo_sb = [wos[:, h * D:(h + 1) * D] for h in range(NH)]
                outT = [p2.tile([P, SC], MDT, tag=f"ot{h}", name=f"ot{h}")
                        for h in range(NH)]

                for c in range(NCH):
                    q0 = c * SC
                    nk = 4 * c + 4          # active sk tiles (causal)
                    for h in range(NH):
                        pts = []
                        offs = []
                        # all-ones stationary -> every psum partition gets
                        # the column sum over sk (broadcast for free)
                        sums_ps = pss.tile([P, SC], FP32, tag="sums")
                        for k in range(nk):
                            # diagonal blocks: sk tile k only attends to
                            # sq >= 128k, i.e. chunk columns [off:512).
                            # f32r matmuls need moving dim >= 256 for the
                            # 1 cyc/row mode, so keep at least 256 columns
                            # (the extra columns are masked -> exp -> 0).
                            off = max(0, (k - 4 * c) * P)
                            if MDT == F32R:
                                off = min(off, SC - 2 * P)
                            sc_ps = ps2.tile([P, SC], FP32, tag="sc", bufs=3)
                            nc.tensor.matmul(
                                sc_ps[:, off:],
                                kt[:, k * P:(k + 1) * P],
                                qt[h][:, q0 + off:q0 + SC],
                                start=True, stop=True,
                            )
                            pt = ptp.tile([P, SC], MDT, tag="pt")
                            if k >= 4 * c:
                                # diagonal block: scores*scale + mask, exp
                                r = k % NCH
                                nc.vector.scalar_tensor_tensor(
                                    sc_ps[:, off:], sc_ps[:, off:], SCALE,
                                    mdiag_sb[r][:, off:],
                                    op0=mybir.AluOpType.mult,
                                    op1=mybir.AluOpType.add)
                                nc.scalar.activation(
                                    pt[:, off:], sc_ps[:, off:],
                                    mybir.ActivationFunctionType.Exp)
                            else:
                                nc.scalar.activation(
                                    pt[:, off:], sc_ps[:, off:],
                                    mybir.ActivationFunctionType.Exp,
                                    scale=SCALE)
                            pts.append(pt)
                            offs.append(off)
                        for k in range(nk):
                            nc.tensor.matmul(
                                sums_ps[:, offs[k]:], ones_sb[:],
                                pts[k][:, offs[k]:],
                                start=(k == 0), stop=(k == nk - 1),
                            )
                        # AV: outT_h [HD, sq] = sum_k V_k^T @ probsT_k
                        av_ps = ps2.tile([P, SC], FP32, tag="av")
                        for k in range(nk):
                            nc.tensor.matmul(
                                av_ps[:, offs[k]:], v_tiles[k][:],
                                pts[k][:, offs[k]:],
                                start=(k == 0), stop=(k == nk - 1),
                            )
                        # normalize: outT[h] = av * (1/sums)
                        recip = nrm.tile([P, SC], FP32, tag="recip")
                        nc.vector.reciprocal(recip[:], sums_ps[:])
                        nc.vector.tensor_mul(outT[h][:], av_ps[:], recip[:])

                    # output projection for this sq chunk; results are
                    # staged in half-slabs (t pairs) and stored with one
                    # batched DMA each on the otherwise-idle gpsimd queue
                    for t in range(SC // P):
                        yslab = yst.tile([P, D], FP32, tag="yslab")
                        for dci in range(NCH):
                            d0 = dci * SC
                            y_ps = ps2.tile([P, SC], FP32, tag="y", bufs=1)
                            for h in range(NH):
                                nc.tensor.matmul(
                                    y_ps[:],
                                    outT[h][:, t * P:(t + 1) * P],
                                    wo_sb[h][:, d0:d0 + SC],
                                    start=(h == 0), stop=(h == NH - 1),
                                )
                            nc.vector.tensor_copy(
                                yslab[:, d0:d0 + SC], y_ps[:])
                        row0 = q0 + t * P
                        nc.gpsimd.dma_start(
                            y_d[row0:row0 + P, :], yslab[:])

    nc.compile()
    return nc


def _host_tables():
    inv_freq = 1.0 / (THETA ** (np.arange(0, HD, 2, dtype=np.float32) / HD))
    t = np.arange(S, dtype=np.float32)
    freqs = t[:, None] * inv_freq[None, :]              # [S, HD/2]
    emb = np.concatenate([freqs, freqs], axis=-1)       # [S, HD]
    cos = np.cos(emb).astype(np.float32)
    sin = np.sin(emb).astype(np.float32)
    cosT = np.ascontiguousarray(cos.T)                  # [HD, S]
    sinT = np.ascontiguousarray(sin.T)
    sinrotT = sinT.copy()
    sinrotT[0:HD // 2] = -sinT[0:HD // 2]
    return cosT, sinrotT


def get_program(mm_mode=MM_MODE, repeat=1):
    key = ("nc", mm_mode, repeat)
    if key not in _CACHE:
        _CACHE[key] = _build_program(mm_mode, repeat)
    return _CACHE[key]


def make_in_maps(x, wq, wk, wv, wo, mask):
    x = np.asarray(x, dtype=np.float32)
    wq = np.asarray(wq, dtype=np.float32)
    wk = np.asarray(wk, dtype=np.float32)
    wv = np.asarray(wv, dtype=np.float32)
    wo = np.asarray(wo, dtype=np.float32)
    mask = np.asarray(mask, dtype=np.float32)

    cosT, sinrotT = _host_tables()
    ident = np.eye(P, dtype=np.float32)
    # maskdiag[r*128+a, b] = mask[0,0, b, r*128+a]; pattern repeats per chunk
    maskdiag = np.ascontiguousarray(mask[0, 0, 0:SC, 0:SC].T)

    xT = [np.ascontiguousarray(x[b].T) for b in range(B)]
    in_maps = []
    for c in range(NCORES):
        b, g = c // NG, c % NG
        qc0 = g * NH * HD
        kc0 = g * HD
        in_maps.append({
            "xT": xT[b],
            "wqg": np.ascontiguousarray(wq[:, qc0:qc0 + NH * HD]),
            "wkg": np.ascontiguousarray(wk[:, kc0:kc0 + HD]),
            "wvg": np.ascontiguousarray(wv[:, kc0:kc0 + HD]),
            "wog": np.ascontiguousarray(wo[qc0:qc0 + NH * HD, :]),
            "cosT": cosT,
            "sinrotT": sinrotT,
            "maskdiag": maskdiag,
            "ident": ident,
            "ones": np.ones((P, P), dtype=np.float32),
        })
    return in_maps


LAST_RESULTS = None


def _make_exec(nc):
    """Mirror run_bass_via_pjrt's multi-core path, but keep the jitted
    executable so repeated (timed) dispatches skip retrace/reload."""
    import jax
    from jax.experimental.shard_map import shard_map
    from jax.sharding import Mesh, PartitionSpec

    from concourse import bass2jax, mybir as _mybir

    bass2jax.install_neuronx_cc_hook()
    partition_name = (
        nc.partition_id_tensor.name if nc.partition_id_tensor else None)
    in_names, out_names, out_avals, zero_outs = [], [], [], []
    for alloc in nc.m.functions[0].allocations:
        if not isinstance(alloc, _mybir.MemoryLocationSet):
            continue
        name = alloc.memorylocations[0].name
        if alloc.kind == "ExternalInput":
            if name != partition_name:
                in_names.append(name)
        elif alloc.kind == "ExternalOutput":
            shape = tuple(alloc.tensor_shape)
            dtype = _mybir.dt.np(alloc.dtype)
            out_names.append(name)
            out_avals.append(jax.core.ShapedArray(shape, dtype))
            zero_outs.append(np.zeros(shape, dtype))
    n_params = len(in_names)
    n_outs = len(out_avals)
    all_in_names = list(in_names) + list(out_names)
    if partition_name is not None:
        all_in_names.append(partition_name)
    donate = tuple(range(n_params, n_params + n_outs))

    def _body(*args):
        operands = list(args)
        if partition_name is not None:
            operands.append(bass2jax.partition_id_tensor())
        outs = bass2jax._bass_exec_p.bind(
            *operands,
            out_avals=tuple(out_avals),
            in_names=tuple(all_in_names),
            out_names=tuple(out_names),
            lowering_input_output_aliases=(),
            sim_require_finite=True,
            sim_require_nnan=True,
            nc=nc,
        )
        return tuple(outs)

    devices = jax.devices()[:NCORES]
    mesh = Mesh(np.asarray(devices), ("core",))
    sharded = jax.jit(
        shard_map(
            _body, mesh=mesh,
            in_specs=(PartitionSpec("core"),) * (n_params + n_outs),
            out_specs=(PartitionSpec("core"),) * n_outs,
            check_rep=False,
        ),
        donate_argnums=donate, keep_unused=True,
    )
    return {
        "fn": sharded, "in_names": in_names, "out_names": out_names,
        "out_avals": out_avals, "zero_outs": zero_outs, "mesh": mesh,
    }


def get_exec(mm_mode=MM_MODE, repeat=1):
    key = ("exec", mm_mode, repeat)
    if key not in _CACHE:
        _CACHE[key] = _make_exec(get_program(mm_mode, repeat))
    return _CACHE[key]


def _concat_inputs(ex, in_maps):
    return [
        np.concatenate([np.asarray(in_maps[c][name]) for c in range(NCORES)],
                       axis=0)
        for name in ex["in_names"]
    ]


def _concat_zeros(ex):
    return [
        np.zeros((NCORES * z.shape[0], *z.shape[1:]), z.dtype)
        for z in ex["zero_outs"]
    ]


def run_on_device(in_maps, mm_mode=MM_MODE, repeat=1):
    """One dispatch; returns per-core output dicts (numpy)."""
    ex = get_exec(mm_mode, repeat)
    out_arrs = ex["fn"](*_concat_inputs(ex, in_maps), *_concat_zeros(ex))
    res = []
    for c in range(NCORES):
        res.append({
            name: np.asarray(out_arrs[i]).reshape(
                NCORES, *ex["out_avals"][i].shape)[c]
            for i, name in enumerate(ex["out_names"])
        })
    return res


def bench(in_maps, iters=5, mm_mode=MM_MODE, repeat=1):
    """Timed repeated dispatch: inputs pre-placed on device, fresh donated
    zero output buffers pre-placed per iteration. Returns list of wall ns."""
    import time

    import jax
    from jax.sharding import NamedSharding, PartitionSpec

    ex = get_exec(mm_mode, repeat)
    sh = NamedSharding(ex["mesh"], PartitionSpec("core"))
    dev_in = [jax.device_put(a, sh) for a in _concat_inputs(ex, in_maps)]
    zsets = [[jax.device_put(z, sh) for z in _concat_zeros(ex)]
             for _ in range(iters + 1)]
    jax.block_until_ready(dev_in)
    jax.block_until_ready(zsets)
    out = ex["fn"](*dev_in, *zsets[0])       # warm-up
    jax.block_until_ready(out)
    times = []
    for i in range(iters):
        t0 = time.perf_counter()
        out = ex["fn"](*dev_in, *zsets[i + 1])
        jax.block_until_ready(out)
        times.append((time.perf_counter() - t0) * 1e9)
    return times


def bench_slope(in_maps, iters=8, mm_mode=MM_MODE, r_hi=4):
    """Per-iteration kernel time via slope: (T(r_hi) - T(1)) / (r_hi - 1).
    Immune to constant dispatch overhead."""
    t1 = bench(in_maps, iters=iters, mm_mode=mm_mode, repeat=1)
    th = bench(in_maps, iters=iters, mm_mode=mm_mode, repeat=r_hi)
    t1m, thm = np.median(t1), np.median(th)
    t1b, thb = min(t1), min(th)
    return {
        "t1": t1, "th": th,
        "exec_ns_median": (thm - t1m) / (r_hi - 1),
        "exec_ns_min": (thb - t1b) / (r_hi - 1),
    }


def kernel(x, wq, wk, wv, wo, mask):
    """Full inputs in, full output out; shards over the 8 NeuronCores."""
    global LAST_RESULTS
    from concourse import bass_utils

    nc = get_program()
    in_maps = make_in_maps(x, wq, wk, wv, wo, mask)
    res = bass_utils.run_bass_kernel_spmd(
        nc, in_maps, core_ids=list(range(NCORES)))
    LAST_RESULTS = res
    out = np.zeros((B, S, D), dtype=np.float32)
    for c in range(NCORES):
        b = c // NG
        out[b] += res.results[c]["y"]
    return out
